# revision 4
# baseline (speedup 1.0000x reference)
"""Trainium2 Bass kernel for nn_NodeUpdateBlock (gnn_message_passing).

Math (per node n):
  out0 = m0 @ W_lin0 * inv_lin + einsum('u,v,uvw->w', f0, attrs, W_tp0) * inv_tp
  out1_m = m1_m @ W_lin1 * inv_lin + einsum('u,v,uvw->w', f1_m, attrs, W_tp1) * inv_tp

Device strategy (per core, nodes sharded 8 ways):
  Everything is computed feature-major ([feat, node]) so the PE contracts
  over the partition dimension with zero on-chip transposes.  The host
  pre-transposes inputs, folds the attrs factor into the streamed operand
  (X planes = feats * attrs outer product), folds the normalization into
  the weights, and transposes the output back at the end.  On device each
  512-node batch is 20 matmuls accumulating into 4 PSUM banks + copyback.
"""

import numpy as np

try:
    import jax as _jax
    _jax.config.update("jax_compilation_cache_dir", "/tmp/jax_neff_cache")
    _jax.config.update("jax_persistent_cache_min_entry_size_bytes", -1)
    _jax.config.update("jax_persistent_cache_min_compile_time_secs", 0)
except Exception:
    pass

import concourse.bass as bass
from concourse import bacc
import concourse.mybir as mybir
import concourse.tile as tile
from concourse.bass_utils import run_bass_kernel_spmd

MUL = 128
K = 4
DIM = 512
N_CORES = 8
N_TOTAL = 100000
NPC = N_TOTAL // N_CORES      # 12500 nodes per core
BATCH = 512
NPAD = 12544                  # 24*512 + 256
IN_PLANES = 20                # m0T, m1T(3), X0T(4), X1T(12)
OUT_PLANES = 4                # out0T, out1T(3)
W_PLANES = 10                 # Wlin0, Wlin1, Wtp0(4), Wtp1(4)

_BATCHES = []
_off = 0
while _off < NPAD:
    _BATCHES.append((_off, min(BATCH, NPAD - _off)))
    _off += BATCH

_CACHE = {}


def _build(dt_in):
    nc = bacc.Bacc("TRN2", target_bir_lowering=False, debug=False,
                   num_devices=N_CORES)
    A = nc.dram_tensor("A", [IN_PLANES, 128, NPAD], dt_in,
                       kind="ExternalInput").ap()
    W = nc.dram_tensor("W", [W_PLANES, 128, 128], dt_in,
                       kind="ExternalInput").ap()
    O = nc.dram_tensor("O", [OUT_PLANES, 128, NPAD], mybir.dt.float32,
                       kind="ExternalOutput").ap()

    with tile.TileContext(nc) as tc:
        with (
            tc.sbuf_pool(name="wpool", bufs=1) as wpool,
            tc.sbuf_pool(name="apool", bufs=3) as apool,
            tc.sbuf_pool(name="opool", bufs=3) as opool,
            tc.psum_pool(name="ppool", bufs=2) as ppool,
        ):
            wtile = wpool.tile([128, W_PLANES, 128], dt_in)
            for j in range(W_PLANES):
                nc.sync.dma_start(out=wtile[:, j, :], in_=W[j])

            for off, nb in _BATCHES:
                atile = apool.tile([128, IN_PLANES, BATCH], dt_in, tag="a")
                for j in range(IN_PLANES):
                    nc.sync.dma_start(out=atile[:, j, :nb],
                                      in_=A[j, :, off:off + nb])

                otile = opool.tile([128, OUT_PLANES, BATCH],
                                   mybir.dt.float32, tag="o")
                # out0T: lin0 + sum_v tp0_v ; out1T_m: lin1_m + sum_v tp1_mv
                for jo in range(OUT_PLANES):
                    ps = ppool.tile([128, BATCH], mybir.dt.float32,
                                    tag=f"ps{jo}")
                    if jo == 0:
                        wlin, acts = 0, [0] + [4 + v for v in range(4)]
                        wtps = [2 + v for v in range(4)]
                    else:
                        m = jo - 1
                        wlin, acts = 1, [1 + m] + [8 + 4 * m + v
                                                   for v in range(4)]
                        wtps = [6 + v for v in range(4)]
                    wseq = [wlin] + wtps
                    for i, (jw, ja) in enumerate(zip(wseq, acts)):
                        nc.tensor.matmul(ps[:, :nb], wtile[:, jw, :],
                                         atile[:, ja, :nb],
                                         start=(i == 0), stop=(i == 4))
                    if jo % 2 == 0:
                        nc.vector.tensor_copy(otile[:, jo, :nb], ps[:, :nb])
                    else:
                        nc.scalar.copy(otile[:, jo, :nb], ps[:, :nb])

                for jo in range(OUT_PLANES):
                    nc.sync.dma_start(out=O[jo, :, off:off + nb],
                                      in_=otile[:, jo, :nb])
    nc.compile()
    return nc


def _prep(m_i, node_feats, node_attrs, W_lin0, W_lin1, W_tp0, W_tp1, np_dt):
    inv_lin = 1.0 / np.sqrt(MUL)
    inv_tp = 1.0 / np.sqrt(MUL * K)
    N = m_i.shape[0]

    Wst = np.empty((W_PLANES, 128, 128), np.float32)
    Wst[0] = W_lin0 * inv_lin
    Wst[1] = W_lin1 * inv_lin
    for v in range(K):
        Wst[2 + v] = W_tp0[:, v, :] * inv_tp
        Wst[6 + v] = W_tp1[:, v, :] * inv_tp
    Wst = Wst.astype(np_dt)

    m0 = m_i[:, :MUL]
    m1 = m_i[:, MUL:].reshape(N, MUL, 3)
    f0 = node_feats[:, :MUL]
    f1 = node_feats[:, MUL:].reshape(N, MUL, 3)

    in_maps = []
    for c in range(N_CORES):
        sl = slice(c * NPC, (c + 1) * NPC)
        A = np.zeros((IN_PLANES, 128, NPAD), np_dt)
        a = node_attrs[sl]                      # [n, 4]
        A[0, :, :NPC] = m0[sl].T
        f0T = np.ascontiguousarray(f0[sl].T)    # [128, n]
        for m in range(3):
            A[1 + m, :, :NPC] = m1[sl, :, m].T
        for v in range(K):
            A[4 + v, :, :NPC] = f0T * a[:, v]
        for m in range(3):
            f1T = np.ascontiguousarray(f1[sl, :, m].T)
            for v in range(K):
                A[8 + 4 * m + v, :, :NPC] = f1T * a[:, v]
        in_maps.append({"A": A, "W": Wst})
    return in_maps


LAST_RESULT = None


def kernel(m_i, node_feats, node_attrs, W_lin0, W_lin1, W_tp0, W_tp1):
    global LAST_RESULT
    np_dt, bass_dt = _CACHE.get("dtype", (np.float32, mybir.dt.float32))

    m_i = np.asarray(m_i, np.float32)
    node_feats = np.asarray(node_feats, np.float32)
    node_attrs = np.asarray(node_attrs, np.float32)

    in_maps = _prep(m_i, node_feats, node_attrs,
                    np.asarray(W_lin0, np.float32),
                    np.asarray(W_lin1, np.float32),
                    np.asarray(W_tp0, np.float32),
                    np.asarray(W_tp1, np.float32), np_dt)

    if "nc" not in _CACHE:
        _CACHE["nc"] = _build(bass_dt)
    nc = _CACHE["nc"]

    import os
    want_trace = bool(os.environ.get("KERNEL_TRACE"))
    try:
        res = run_bass_kernel_spmd(
            nc, in_maps, core_ids=list(range(N_CORES)), trace=want_trace,
        )
    except ModuleNotFoundError:
        res = run_bass_kernel_spmd(
            nc, in_maps, core_ids=list(range(N_CORES)), trace=False,
        )
    LAST_RESULT = res

    N = m_i.shape[0]
    out = np.empty((N, DIM), np.float32)
    for c in range(N_CORES):
        O = res.results[c]["O"]                 # [4, 128, NPAD] f32
        sl = slice(c * NPC, (c + 1) * NPC)
        out[sl, :MUL] = O[0, :, :NPC].T
        out1 = np.stack([O[1 + m, :, :NPC] for m in range(3)], axis=-1)
        out[sl, MUL:] = out1.transpose(1, 0, 2).reshape(NPC, MUL * 3)
    return out


# revision 5
# speedup vs baseline: 2.1420x; 2.1420x over previous
"""Trainium2 Bass kernel for nn_NodeUpdateBlock (gnn_message_passing).

Math (per node n):
  out0   = m0 @ W_lin0 * inv_lin   + einsum('u,v,uvw->w', f0, attrs, W_tp0) * inv_tp
  out1_m = m1_m @ W_lin1 * inv_lin + einsum('u,v,uvw->w', f1_m, attrs, W_tp1) * inv_tp

Strategy: nodes sharded 8 ways (data parallel), weights replicated.
Everything on-device is computed feature-major ([feat, node]) so the PE
contracts over the partition dimension with zero on-chip transposes.
The host pre-transposes inputs, folds the attrs factor into the streamed
operand (X planes = feats * attrs outer product), folds the path
normalizations into the weights, casts to bf16 (PSUM accumulation stays
fp32), and transposes the output back at the end.  Each 512-node batch
is one 2.5MB DMA in, 20 matmuls accumulating into 4 PSUM banks, 4 PSUM
copybacks and one DMA out.  The layout interleaves all 20 input planes
per node-block so every DMA moves 20KB contiguous per partition.
"""

import numpy as np
import ml_dtypes

try:
    import jax as _jax
    _jax.config.update("jax_compilation_cache_dir", "/tmp/jax_neff_cache")
    _jax.config.update("jax_persistent_cache_min_entry_size_bytes", -1)
    _jax.config.update("jax_persistent_cache_min_compile_time_secs", 0)
except Exception:
    pass

import concourse.bass as bass
from concourse import bacc
import concourse.mybir as mybir
import concourse.tile as tile
from concourse.bass_utils import run_bass_kernel_spmd

MUL = 128
K = 4
DIM = 512
N_CORES = 8
N_TOTAL = 100000
NPC = N_TOTAL // N_CORES      # 12500 nodes per core
BATCH = 512
NB = 25                       # batches per core
NPAD = NB * BATCH             # 12800 (300 nodes of zero padding)
IN_PLANES = 20                # m0T, m1T(3), X0T(4), X1T(12)
OUT_PLANES = 4                # out0T, out1T(3)
W_PLANES = 10                 # Wlin0, Wlin1, Wtp0(4), Wtp1(4)

_CACHE = {}


def _build(dt_in, dt_out):
    nc = bacc.Bacc("TRN2", target_bir_lowering=False, debug=False,
                   num_devices=N_CORES)
    A = nc.dram_tensor("A", [128, NB, IN_PLANES, BATCH], dt_in,
                       kind="ExternalInput").ap()
    W = nc.dram_tensor("W", [W_PLANES, 128, 128], dt_in,
                       kind="ExternalInput").ap()
    O = nc.dram_tensor("O", [128, NB, OUT_PLANES, BATCH], dt_out,
                       kind="ExternalOutput").ap()

    with tile.TileContext(nc) as tc:
        with (
            tc.sbuf_pool(name="wpool", bufs=1) as wpool,
            tc.sbuf_pool(name="apool", bufs=4) as apool,
            tc.sbuf_pool(name="opool", bufs=4) as opool,
            tc.psum_pool(name="ppool", bufs=2) as ppool,
        ):
            wtile = wpool.tile([128, W_PLANES, 128], dt_in)
            for j in range(W_PLANES):
                nc.sync.dma_start(out=wtile[:, j, :], in_=W[j])

            for b in range(NB):
                atile = apool.tile([128, IN_PLANES, BATCH], dt_in, tag="a")
                nc.sync.dma_start(out=atile[:], in_=A[:, b])

                otile = opool.tile([128, OUT_PLANES, BATCH], dt_out, tag="o")
                # out0T: lin0 + sum_v tp0_v ; out1T_m: lin1_m + sum_v tp1_mv
                for jo in range(OUT_PLANES):
                    ps = ppool.tile([128, BATCH], mybir.dt.float32,
                                    tag=f"ps{jo}")
                    if jo == 0:
                        wseq = [0] + [2 + v for v in range(4)]
                        acts = [0] + [4 + v for v in range(4)]
                    else:
                        m = jo - 1
                        wseq = [1] + [6 + v for v in range(4)]
                        acts = [1 + m] + [8 + 4 * m + v for v in range(4)]
                    for i, (jw, ja) in enumerate(zip(wseq, acts)):
                        nc.tensor.matmul(ps[:], wtile[:, jw, :],
                                         atile[:, ja, :],
                                         start=(i == 0), stop=(i == 4))
                    if jo % 2 == 0:
                        nc.vector.tensor_copy(otile[:, jo, :], ps[:])
                    else:
                        nc.scalar.copy(otile[:, jo, :], ps[:])

                nc.sync.dma_start(out=O[:, b], in_=otile[:])
    nc.compile()
    return nc


def _prep(m_i, node_feats, node_attrs, W_lin0, W_lin1, W_tp0, W_tp1, np_dt):
    inv_lin = 1.0 / np.sqrt(MUL)
    inv_tp = 1.0 / np.sqrt(MUL * K)
    N = m_i.shape[0]

    Wst = np.empty((W_PLANES, 128, 128), np.float32)
    Wst[0] = W_lin0 * inv_lin
    Wst[1] = W_lin1 * inv_lin
    for v in range(K):
        Wst[2 + v] = W_tp0[:, v, :] * inv_tp
        Wst[6 + v] = W_tp1[:, v, :] * inv_tp
    Wst = Wst.astype(np_dt)

    m0 = m_i[:, :MUL]
    m1 = m_i[:, MUL:].reshape(N, MUL, 3)
    f0 = node_feats[:, :MUL]
    f1 = node_feats[:, MUL:].reshape(N, MUL, 3)

    in_maps = []
    for c in range(N_CORES):
        sl = slice(c * NPC, (c + 1) * NPC)
        a = node_attrs[sl]                      # [n, 4]
        # planes[p] : [128, NPC] feature-major slab for this core
        planes = np.zeros((IN_PLANES, 128, NPAD), np.float32)
        planes[0, :, :NPC] = m0[sl].T
        f0T = np.ascontiguousarray(f0[sl].T)    # [128, n]
        for m in range(3):
            planes[1 + m, :, :NPC] = m1[sl, :, m].T
        for v in range(K):
            planes[4 + v, :, :NPC] = f0T * a[:, v]
        for m in range(3):
            f1T = np.ascontiguousarray(f1[sl, :, m].T)
            for v in range(K):
                planes[8 + 4 * m + v, :, :NPC] = f1T * a[:, v]
        # [p, 128, NB, BATCH] -> [128, NB, p, BATCH] node-block interleave
        A = np.ascontiguousarray(
            planes.reshape(IN_PLANES, 128, NB, BATCH).transpose(1, 2, 0, 3)
        ).astype(np_dt)
        in_maps.append({"A": A, "W": Wst})
    return in_maps


LAST_RESULT = None


def kernel(m_i, node_feats, node_attrs, W_lin0, W_lin1, W_tp0, W_tp1):
    global LAST_RESULT
    np_dt = _CACHE.get("np_dt", ml_dtypes.bfloat16)
    bass_dt = _CACHE.get("bass_dt", mybir.dt.bfloat16)
    np_dt_out = _CACHE.get("np_dt_out", ml_dtypes.bfloat16)
    bass_dt_out = _CACHE.get("bass_dt_out", mybir.dt.bfloat16)

    m_i = np.asarray(m_i, np.float32)
    node_feats = np.asarray(node_feats, np.float32)
    node_attrs = np.asarray(node_attrs, np.float32)

    in_maps = _prep(m_i, node_feats, node_attrs,
                    np.asarray(W_lin0, np.float32),
                    np.asarray(W_lin1, np.float32),
                    np.asarray(W_tp0, np.float32),
                    np.asarray(W_tp1, np.float32), np_dt)

    if "nc" not in _CACHE:
        _CACHE["nc"] = _build(bass_dt, bass_dt_out)
    nc = _CACHE["nc"]

    import os
    want_trace = bool(os.environ.get("KERNEL_TRACE"))
    try:
        res = run_bass_kernel_spmd(
            nc, in_maps, core_ids=list(range(N_CORES)), trace=want_trace,
        )
    except ModuleNotFoundError:
        res = run_bass_kernel_spmd(
            nc, in_maps, core_ids=list(range(N_CORES)), trace=False,
        )
    LAST_RESULT = res

    N = m_i.shape[0]
    out = np.empty((N, DIM), np.float32)
    for c in range(N_CORES):
        O = res.results[c]["O"]                 # [128, NB, 4, BATCH]
        Opl = np.ascontiguousarray(
            O.transpose(2, 0, 1, 3), np.float32
        ).reshape(OUT_PLANES, 128, NPAD)[:, :, :NPC]
        sl = slice(c * NPC, (c + 1) * NPC)
        out[sl, :MUL] = Opl[0].T
        out1 = np.stack([Opl[1 + m] for m in range(3)], axis=-1)
        out[sl, MUL:] = out1.transpose(1, 0, 2).reshape(NPC, MUL * 3)
    return out
